# revision 16
# baseline (speedup 1.0000x reference)
"""GATv2 layer kernel for 8 Trainium2 NeuronCores.

Mathematical structure exploited: in this GATv2 variant the value vectors are
gathered at the *destination* node (Vv = node_feats[dest] @ W_v + b_v), so for
every destination node d the aggregation

    out[d] = sum_{e: dest_e = d} alpha_e * (node_feats[d] @ W_v + b_v)
           = (node_feats[d] @ W_v + b_v) * sum_e alpha_e
           = (node_feats[d] @ W_v + b_v) * [deg_in(d) > 0]

because the softmax weights alpha sum to exactly 1 within each destination
segment (and the sum is empty for isolated nodes). Q/K/edge_feats/a_w only
reweight terms inside a softmax that cancels entirely. Verified against the
reference: max relative error ~2.6e-7 (pure fp32 rounding).

Device computation per core c (nodes sharded 6272/core):
  feature-major matmul out.T = W_aug.T @ x_aug.T (bias folded in via a ones
  row of x_aug, weight stationary on the PE), multiplied by the per-node
  presence mask broadcast across the 64 output features with a K=1 matmul.
The presence bitmap (50 KB) is folded on the host during input sharding; the
node features, weights and output (the dominant data volume) stream through
the NeuronCores.

Sync-wait discipline: this container's walrus build allows only ONE semaphore
wait per instruction and Tile's sem assignment is not transitive, so every
cross-engine dependency is pre-observed by a cheap same-engine op (dummy
matmuls on PE, tiny memsets on DVE, NoOps on SP) that each carry exactly one
wait, and producer-dependent stores go through SWDGE (Pool) behind Pool-side
observers. A final SP NoOp chain observes all async completions so the
kernel-tail drain needs no waits of its own.
"""
import numpy as np

import concourse.bass as bass
import concourse.mybir as mybir
import concourse.tile as tile
from concourse.bass_utils import run_bass_kernel_spmd
from concourse.tile_rust import add_dep_helper

V, E = 50000, 800000
D_IN, D_OUT = 64, 64
NCORES = 8
P = 128
SHARD = 6272                # nodes per core
VPAD = SHARD * NCORES       # 50176
MM = 512                    # node columns per matmul chunk

_cache = {}


def _build():
    nc = bass.Bass()
    xt = nc.dram_tensor("xt", [D_IN + 1, SHARD], mybir.dt.float32, kind="ExternalInput")
    w = nc.dram_tensor("w", [D_IN + 1, D_OUT], mybir.dt.float32, kind="ExternalInput")
    mrow = nc.dram_tensor("m", [1, SHARD], mybir.dt.float32, kind="ExternalInput")
    out_t = nc.dram_tensor("out_t", [D_OUT, SHARD], mybir.dt.float32, kind="ExternalOutput")

    with tile.TileContext(nc) as tc:
        with (
            tc.tile_pool(name="const", bufs=1) as const,
            tc.tile_pool(name="xin", bufs=13) as xin,
            tc.tile_pool(name="osb", bufs=13) as osb,
            tc.tile_pool(name="po", bufs=3, space="PSUM") as po,
            tc.tile_pool(name="pm", bufs=2, space="PSUM") as pm,
            tc.tile_pool(name="pd", bufs=1, space="PSUM") as pd,
        ):
            async_insts = []

            mask_row = const.tile([1, SHARD], mybir.dt.float32)
            i_m = nc.sync.dma_start(out=mask_row[:], in_=mrow[:])
            w_sb = const.tile([D_IN + 1, D_OUT], mybir.dt.float32)
            i_w = nc.sync.dma_start(out=w_sb[:], in_=w[:])
            ones_col = const.tile([1, D_OUT], mybir.dt.float32)
            nc.vector.memset(ones_col[:], 1.0)

            # warm PE's clock: dummy matmuls observing w, mask and ones_col
            dummy = pd.tile([D_OUT, 1], mybir.dt.float32)
            mw = nc.tensor.matmul(dummy[:], lhsT=w_sb[:], rhs=w_sb[:, 0:1], start=True, stop=True)
            add_dep_helper(mw.ins, i_w.ins, True, "warm PE: observe w dma")
            dummy2 = pd.tile([D_OUT, 1], mybir.dt.float32)
            nc.tensor.matmul(dummy2[:], lhsT=ones_col[:], rhs=ones_col[:, 0:1], start=True, stop=True)
            m3 = nc.tensor.matmul(dummy2[:], lhsT=ones_col[:], rhs=ones_col[:, 0:1], start=True, stop=True)
            add_dep_helper(m3.ins, i_m.ins, True, "warm PE: observe mask dma")

            mm_bounds = list(range(0, SHARD, MM)) + [SHARD]
            spans = list(zip(mm_bounds[:-1], mm_bounds[1:]))

            # materialize the mask broadcast across the 64 feature partitions
            # (K=1 matmul) so the masked multiply reads only one PSUM operand
            mask_bT = const.tile([D_OUT, SHARD], mybir.dt.float32)
            last_mcopy = None
            for a, b in spans:
                n = b - a
                m_pT = pm.tile([D_OUT, MM], mybir.dt.float32, tag="mpt")
                nc.tensor.matmul(m_pT[:, :n], lhsT=ones_col[:], rhs=mask_row[:, a:b], start=True, stop=True)
                last_mcopy = nc.vector.tensor_copy(out=mask_bT[:, a:b], in_=m_pT[:, :n])

            scratch = const.tile([1, 16], mybir.dt.float32)
            scratch2 = const.tile([1, 16], mybir.dt.float32)
            tts = []
            for j, (a, b) in enumerate(spans):
                n = b - a
                xt_sb = xin.tile([D_IN + 1, MM], mybir.dt.float32, tag="xt")
                xdma = nc.sync.dma_start(out=xt_sb[:, :n], in_=xt[:, a:b])

                # PE observers: absorb the matmul's cross-engine deps (1 wait each)
                ob1 = nc.tensor.matmul(dummy[:], lhsT=w_sb[:], rhs=w_sb[:, 0:1], start=True, stop=True)
                add_dep_helper(ob1.ins, xdma.ins, True, "PE observes xt load")
                tgt = tts[j - 3] if j >= 3 else last_mcopy
                ob2 = nc.tensor.matmul(dummy[:], lhsT=w_sb[:], rhs=w_sb[:, 0:1], start=True, stop=True)
                add_dep_helper(ob2.ins, tgt.ins, True, "PE observes DVE tick")

                o_pT = po.tile([D_OUT, MM], mybir.dt.float32, tag="opt")
                mm = nc.tensor.matmul(o_pT[:, :n], lhsT=w_sb[:], rhs=xt_sb[:, :n], start=True, stop=True)
                add_dep_helper(mm.ins, ob1.ins, False, "after PE observer 1")
                add_dep_helper(mm.ins, ob2.ins, False, "after PE observer 2")

                # DVE observer: absorb the masked-multiply's PE dep
                dob = nc.vector.memset(scratch2[:, j : j + 1], 0.0)
                add_dep_helper(dob.ins, mm.ins, True, "DVE observes matmul")

                o_sb = osb.tile([D_OUT, MM], mybir.dt.float32, tag="osb")
                tt = nc.vector.tensor_tensor(
                    out=o_sb[:, :n], in0=o_pT[:, :n], in1=mask_bT[:, a:b], op=mybir.AluOpType.mult
                )
                add_dep_helper(tt.ins, dob.ins, False, "after DVE observer")
                tts.append(tt)

                # store via SWDGE behind a Pool observer (HWDGE DMA waits live
                # on DMA-lane procs and cannot be pre-observed there)
                ob = nc.gpsimd.memset(scratch[:, j : j + 1], 0.0)
                add_dep_helper(ob.ins, tt.ins, True, "Pool observes masked tile")
                st = nc.gpsimd.dma_start(out=out_t[:, a:b], in_=o_sb[:, :n])
                async_insts.extend([xdma, st, tt, ob2, ob])

            # final SP chain: observe every async completion with one wait per
            # NoOp so the kernel-tail drain needs no new waits of its own
            async_insts.extend([i_w, i_m])
            chain_prev = None
            for dep in async_insts:
                nn = nc.sync.nop()
                add_dep_helper(nn.ins, dep.ins, True, "tail observe")
                if chain_prev is not None:
                    add_dep_helper(nn.ins, chain_prev.ins, False, "tail chain order")
                chain_prev = nn
    return nc


def _get_nc():
    if "nc" not in _cache:
        _cache["nc"] = _build()
    return _cache["nc"]


def _stage(node_feats, W_v, b_v, edge_index):
    x_aug_t = np.ones((D_IN + 1, VPAD), dtype=np.float32)
    x_aug_t[:D_IN, :V] = np.asarray(node_feats, dtype=np.float32).T
    x_aug_t[:D_IN, V:] = 0.0
    w_aug = np.concatenate(
        [np.asarray(W_v, np.float32), np.asarray(b_v, np.float32)[None, :]], axis=0
    )
    dest = np.asarray(edge_index)[1].astype(np.int64)
    flag = np.zeros(VPAD, dtype=np.float32)
    flag[np.clip(dest, 0, V - 1)] = 1.0

    in_maps = []
    for c in range(NCORES):
        in_maps.append(
            {
                "xt": np.ascontiguousarray(x_aug_t[:, SHARD * c : SHARD * (c + 1)]),
                "w": w_aug,
                "m": np.ascontiguousarray(flag[None, SHARD * c : SHARD * (c + 1)]),
            }
        )
    return in_maps


def _run(in_maps, **kwargs):
    nc = _get_nc()
    return run_bass_kernel_spmd(nc, in_maps, core_ids=list(range(NCORES)), **kwargs)


def kernel(
    node_feats, edge_feats, edge_index, W_q, b_q, W_k, b_k, W_v, b_v, W_e, b_e, a_w, a_b
) -> np.ndarray:
    in_maps = _stage(node_feats, W_v, b_v, edge_index)
    res = _run(in_maps)
    full_t = np.concatenate([res.results[c]["out_t"] for c in range(NCORES)], axis=1)
    return np.ascontiguousarray(full_t[:, :V].T).astype(np.float32)
